# revision 22
# baseline (speedup 1.0000x reference)
"""Maxwell viscoelastic model (linear recurrence scan) on 8 Trainium2 NeuronCores.

Math (per trajectory, T timesteps):
    a_n = 1 - 2*dt_n
    gamma_n = a_n*gamma_{n-1} + 2*dt_n*eps_n,  gamma_0 = 0
    sigma_n = 2.5*eps_n - 2*gamma_n

Strategy: batch-shard 4096 trajectories across 8 cores (512 each); all I/O in
fp16 (the 2e-2 rel-err budget dwarfs fp16 quantization at ~1e-3), which halves
HBM traffic to ~12.6 MB/core and doubles DVE elementwise throughput.  The host
packs per-core rows of [eps plane | dm4 plane] where dm4 = -4*dt, and unpacks
the fp16 sigma output back to f32.

Per chunk q (L=2048 timesteps) the recurrence g = -2*gamma runs as
    g = a*g + (dm4*eps)        [tensor_tensor_scan, ~2 cycles/elem]
    sigma = 2.5*eps + g        [fp16 2x tensor_tensor]
with the engine split:
  SYNC   chunk loads + chunk stores (one HWDGE ring)
  ACT    a = 1 + 0.5*dm4 -> PSUM f32;  e25 = 2.5*eps -> SBUF fp16
  POOL   bneg = dm4 * eps -> SBUF fp16 for chunks q>=1  (GpSimd)
  DVE    scan(a[PSUM], bneg[SBUF]) -> g[SBUF fp16]; sigma = e25 + g

Port discipline: DVE's 2nd SBUF read port is shared with GpSimd.  The scan
keeps data0 in PSUM (1 SBUF read), so GpSimd's bneg(q) runs collision-free
inside the scan(q-1) window -- a phase-lock that holds from chunk 1 on because
bneg(q) is gated on sigma(q-2) and fits inside the 4.4us scan.  bneg(0) is
computed by DVE itself (a single fp16 2x TT that finishes before GpSimd's
first op, which is gated on a(0) via act_seq).

Raw bass (no TileContext): explicit semaphores, one per (buffer slot, chunk)
for DMA completion since two DMAs on one ring can complete out of order.
"""

import numpy as np

import concourse.bass as bass
import concourse.mybir as mybir
from concourse.bass_utils import run_bass_kernel_spmd

N_CORES = 8
P = 128                      # SBUF partitions
T = 4096                     # timesteps
CH = 2                       # chunks per tile
L = T // CH                  # 2048 timesteps per chunk
ROW = 2 * T                  # packed row: eps[T] | dm4[T]
XT_BUFS = 3                  # xt tile ring depth

f32 = mybir.dt.float32
f16 = mybir.dt.float16


def build_nc(b_shard: int) -> bass.Bass:
    nc = bass.Bass()
    x = nc.dram_tensor("x", [b_shard, ROW], f16, kind="ExternalInput")
    y = nc.dram_tensor("y", [b_shard, T], f16, kind="ExternalOutput")
    n_tiles = b_shard // P
    assert n_tiles * P == b_shard

    xr = x.rearrange("(n p) f -> n p f", p=P)   # [n_tiles, 128, ROW]
    yr = y.rearrange("(n p) t -> n p t", p=P)   # [n_tiles, 128, T]
    mult = mybir.AluOpType.mult
    add = mybir.AluOpType.add
    Copy = mybir.ActivationFunctionType.Copy
    NQ = CH * n_tiles

    def eps_s(c):
        return slice(L * c, L * (c + 1))

    def dm4_s(c):
        return slice(T + L * c, T + L * (c + 1))

    def chunk_s(c):
        return slice(L * c, L * (c + 1))

    with (
        nc.sbuf_tensor("xt0", [P, ROW], f16) as xt0,
        nc.sbuf_tensor("xt1", [P, ROW], f16) as xt1,
        nc.sbuf_tensor("xt2", [P, ROW], f16) as xt2,
        nc.sbuf_tensor("bneg0", [P, L], f16) as bneg0,
        nc.sbuf_tensor("bneg1", [P, L], f16) as bneg1,
        nc.sbuf_tensor("e250", [P, L], f16) as e250,
        nc.sbuf_tensor("e251", [P, L], f16) as e251,
        nc.sbuf_tensor("g0", [P, L], f16) as g0,
        nc.sbuf_tensor("g1", [P, L], f16) as g1,
        nc.sbuf_tensor("sig0", [P, T], f16) as sig0,
        nc.sbuf_tensor("sig1", [P, T], f16) as sig1,
        nc.psum_tensor("pa0", [P, L], f32) as pa0,
        nc.psum_tensor("pa1", [P, L], f32) as pa1,
        nc.semaphore("act_seq") as act_seq,    # +1 per ACT op (a -> 2q+1, e25 -> 2q+2)
        nc.semaphore("pool_seq") as pool_seq,  # +1 per bneg chunk (POOL); bneg(q) -> q
        nc.semaphore("dve_seq") as dve_seq,    # +1 per sigma (DVE); sigma(q) -> q+1
        nc.semaphore("dm40") as dm40,          # chunk-0 dm4 half-1 arrival
        nc.semaphore("dm4b") as dm4b,          # chunk-0 dm4 half-2 arrival
        nc.semaphore("eps0a") as eps0a,        # chunk-0 eps half-1 arrival
        nc.Block(no_gpsimd_drain=True) as block,
    ):
        sem_in = [
            [nc.alloc_semaphore(f"in{s}_{c}") for c in range(CH)]
            for s in range(XT_BUFS)
        ]
        sem_out = [
            [nc.alloc_semaphore(f"out{s}_{c}") for c in range(CH)] for s in range(2)
        ]
        xt = [xt0, xt1, xt2]
        bneg = [bneg0, bneg1]
        e25 = [e250, e251]
        g = [g0, g1]
        sig = [sig0, sig1]
        pa = [pa0, pa1]

        def in_target(i, c):
            """(sem, value) meaning chunk (i, c)'s eps+dm4 both arrived."""
            if i % XT_BUFS == 0 and c == 0:
                # tile-0 chunk-0 sent 3 of its 4 quarter-DMAs to dedicated sems
                return sem_in[0][0], 16 + 32 * (i // XT_BUFS)
            return sem_in[i % XT_BUFS][c], 32 * (i // XT_BUFS + 1)

        H = L // 2

        @block.sync
        def _(sync):
            # chunk (0,0) in quarter-loads (128KB): small transfers complete
            # (incl. the ~2us receipt) much sooner, so ACT/DVE start earlier.
            # Each quarter has its own sem -- completion order is not FIFO.
            d0, d1 = dm4_s(0).start, dm4_s(0).stop
            e0, e1 = eps_s(0).start, eps_s(0).stop
            sync.dma_start(xt[0][:, d0 : d0 + H], xr[0][:, d0 : d0 + H]).then_inc(
                dm40, 16)
            sync.dma_start(xt[0][:, e0 : e0 + H], xr[0][:, e0 : e0 + H]).then_inc(
                eps0a, 16)
            sync.dma_start(xt[0][:, d0 + H : d1], xr[0][:, d0 + H : d1]).then_inc(
                dm4b, 16)
            sync.dma_start(xt[0][:, e0 + H : e1], xr[0][:, e0 + H : e1]).then_inc(
                sem_in[0][0], 16)
            for i in range(n_tiles):
                for c in range(CH):
                    if i == 0 and c == 0:
                        continue
                    q = CH * i + c
                    if i >= XT_BUFS:
                        # xt slot chunk reuse: sigma(i-XT_BUFS, c) transitively
                        # implies every reader of that chunk finished.
                        sync.wait_ge(dve_seq, CH * (i - XT_BUFS) + c + 1)
                    sync.dma_start(
                        xt[i % XT_BUFS][:, eps_s(c)], xr[i][:, eps_s(c)]
                    ).then_inc(sem_in[i % XT_BUFS][c], 16)
                    sync.dma_start(
                        xt[i % XT_BUFS][:, dm4_s(c)], xr[i][:, dm4_s(c)]
                    ).then_inc(sem_in[i % XT_BUFS][c], 16)
            NQ = CH * n_tiles
            for i in range(n_tiles):
                for c in range(CH):
                    q = CH * i + c
                    if q == NQ - 1:
                        # final chunk stored in halves right behind the two
                        # half-sigmas -- shortens the kernel tail
                        s0_, s1_ = chunk_s(c).start, chunk_s(c).stop
                        sync.wait_ge(dve_seq, NQ)
                        sync.dma_start(
                            yr[i][:, s0_ : s0_ + H], sig[i % 2][:, s0_ : s0_ + H]
                        ).then_inc(sem_out[i % 2][c], 16)
                        sync.wait_ge(dve_seq, NQ + 1)
                        sync.dma_start(
                            yr[i][:, s0_ + H : s1_], sig[i % 2][:, s0_ + H : s1_]
                        ).then_inc(sem_out[i % 2][c], 16)
                        continue
                    sync.wait_ge(dve_seq, q + 1)     # sigma(q) done
                    sync.dma_start(
                        yr[i][:, chunk_s(c)], sig[i % 2][:, chunk_s(c)]
                    ).then_inc(sem_out[i % 2][c], 16)
            for c in range(CH):
                extra = 16 if c == CH - 1 else 0  # split final store
                sync.wait_ge(sem_out[0][c], 16 * ((n_tiles + 1) // 2)
                             + (extra if (n_tiles - 1) % 2 == 0 else 0))
                if n_tiles >= 2:
                    sync.wait_ge(sem_out[1][c], 16 * (n_tiles // 2)
                                 + (extra if (n_tiles - 1) % 2 == 1 else 0))

        @block.scalar
        def _(scalar):
            # dummy activation before any waits: hoists the lazy ACT_TABLE_LOAD
            # (~1.3us) into the idle preamble instead of after the first load
            scalar.activation(
                e25[0][:, 0:1], e25[0][:, 0:1], Copy, bias=0.0, scale=0.0,
            )
            # act_seq counts: chunk 0 = {a-h1:1, a-h2:2, e25:3};
            # chunk q>=1 = {a:2q+2, e25:2q+3}
            for i in range(n_tiles):
                for c in range(CH):
                    q = CH * i + c
                    if i == 0 and c == 0:
                        d0 = dm4_s(0).start
                        scalar.wait_ge(dm40, 16)
                        scalar.activation(
                            pa[0][:, 0:H], xt[0][:, d0 : d0 + H],
                            Copy, bias=1.0, scale=0.5,
                        ).then_inc(act_seq, 1)
                        scalar.wait_ge(dm4b, 16)
                        scalar.activation(
                            pa[0][:, H:L], xt[0][:, d0 + H : d0 + L],
                            Copy, bias=1.0, scale=0.5,
                        ).then_inc(act_seq, 1)
                        scalar.wait_ge(eps0a, 16)
                        scalar.wait_ge(sem_in[0][0], 16)
                        scalar.activation(
                            e25[0][:], xt[0][:, eps_s(0)],
                            Copy, bias=0.0, scale=2.5,
                        ).then_inc(act_seq, 1)
                        continue
                    s, v = in_target(i, c)
                    scalar.wait_ge(s, v)
                    if q >= 2:
                        # e25/pa slot WAR: sigma(q-2) read e25 and followed
                        # scan(q-2), the pa reader.
                        scalar.wait_ge(dve_seq, q - 1)
                    # a = 1 + 0.5*dm4 -> PSUM f32   (a first: it gates scan(q))
                    scalar.activation(
                        pa[q % 2][:], xt[i % XT_BUFS][:, dm4_s(c)],
                        Copy, bias=1.0, scale=0.5,
                    ).then_inc(act_seq, 1)
                    # e25 = 2.5*eps -> SBUF fp16
                    scalar.activation(
                        e25[q % 2][:], xt[i % XT_BUFS][:, eps_s(c)],
                        Copy, bias=0.0, scale=2.5,
                    ).then_inc(act_seq, 1)

        @block.gpsimd
        def _(gpsimd):
            # chunk 0's bneg runs on DVE; GpSimd starts at chunk 1, gated on
            # a(0) so its first op never overlaps DVE's 2-port bneg(0) TT.
            for i in range(n_tiles):
                for c in range(CH):
                    if i == 0 and c == 0:
                        continue
                    q = CH * i + c
                    s, v = in_target(i, c)
                    gpsimd.wait_ge(s, v)
                    if q == 1:
                        # DVE's 2-port bneg(0) half-TTs finished (pool_seq 2)
                        gpsimd.wait_ge(pool_seq, 2)
                    if q >= 2:
                        # bneg slot WAR: sigma(q-2) followed scan(q-2)
                        gpsimd.wait_ge(dve_seq, q - 1)
                    # bneg = dm4 * eps -> SBUF fp16   (pool_seq: bneg(q) -> q+2,
                    # counts 1-2 are DVE's own bneg(0) halves)
                    gpsimd.tensor_tensor(
                        bneg[q % 2][:],
                        xt[i % XT_BUFS][:, dm4_s(c)],
                        xt[i % XT_BUFS][:, eps_s(c)],
                        mult,
                    ).then_inc(pool_seq, 1)

        @block.vector
        def _(vector):
            NQ = CH * n_tiles
            d0 = dm4_s(0).start
            e0 = eps_s(0).start
            # chunk 0 ramp in halves: bneg on DVE itself (fp16 2x TT), scans
            # chained -- the second TT doubles as the intervening op that
            # keeps scan-h2's `initial` read off scan-h1's in-flight write.
            vector.wait_ge(dm40, 16)
            vector.wait_ge(eps0a, 16)
            vector.tensor_tensor(
                bneg[0][:, 0:H], xt[0][:, d0 : d0 + H], xt[0][:, e0 : e0 + H],
                mult,
            ).then_inc(pool_seq, 1)
            vector.wait_ge(act_seq, 1)           # a-h1 in PSUM
            vector.tensor_tensor_scan(
                g[0][:, 0:H], pa[0][:, 0:H], bneg[0][:, 0:H], 0.0, mult, add,
            )
            vector.wait_ge(dm4b, 16)
            vector.wait_ge(sem_in[0][0], 16)
            vector.tensor_tensor(
                bneg[0][:, H:L], xt[0][:, d0 + H : d0 + L],
                xt[0][:, e0 + H : e0 + L], mult,
            ).then_inc(pool_seq, 1)
            vector.wait_ge(act_seq, 2)           # a-h2 in PSUM
            vector.tensor_tensor_scan(
                g[0][:, H:L], pa[0][:, H:L], bneg[0][:, H:L],
                g[0][:, H - 1 : H], mult, add,
            )
            vector.wait_ge(act_seq, 3)           # e25(0) in SBUF
            vector.tensor_tensor(
                sig[0][:, chunk_s(0)], e25[0][:], g[0][:], add,
            ).then_inc(dve_seq, 1)

            def act_a(q):
                return 2 * q + 2

            for i in range(n_tiles):
                for c in range(CH):
                    q = CH * i + c
                    if i == 0 and c == 0:
                        continue
                    vector.wait_ge(act_seq, act_a(q))    # a(q) in PSUM
                    vector.wait_ge(pool_seq, q + 2)      # bneg(q) in SBUF
                    init = g[(q - 1) % 2][:, L - 1 : L] if c != 0 else 0.0
                    # g = a*g + bneg  (g slot WAR: sigma(q-2) preceded on DVE)
                    vector.tensor_tensor_scan(
                        g[q % 2][:], pa[q % 2][:], bneg[q % 2][:],
                        init, mult, add,
                    )
                    vector.wait_ge(act_seq, act_a(q) + 1)  # e25(q) in SBUF
                    if i >= 2:
                        # sig slot WAR: store(i-2, c) completed
                        vector.wait_ge(
                            sem_out[i % 2][c], 16 * ((i - 2) // 2 + 1)
                        )
                    if q == NQ - 1:
                        # final sigma in halves so the first half-store can
                        # start while the second half computes
                        vector.tensor_tensor(
                            sig[i % 2][:, chunk_s(c).start : chunk_s(c).start + H],
                            e25[q % 2][:, 0:H], g[q % 2][:, 0:H], add,
                        ).then_inc(dve_seq, 1)
                        vector.tensor_tensor(
                            sig[i % 2][:, chunk_s(c).start + H : chunk_s(c).stop],
                            e25[q % 2][:, H:L], g[q % 2][:, H:L], add,
                        ).then_inc(dve_seq, 1)
                        continue
                    # sigma = e25 + g   (fp16 2x tensor_tensor)
                    vector.tensor_tensor(
                        sig[i % 2][:, chunk_s(c)], e25[q % 2][:], g[q % 2][:], add,
                    ).then_inc(dve_seq, 1)

    return nc


_NC_CACHE: dict = {}


def _get_nc(b_shard: int) -> bass.Bass:
    if b_shard not in _NC_CACHE:
        _NC_CACHE[b_shard] = build_nc(b_shard)
    return _NC_CACHE[b_shard]


def _pack(x: np.ndarray) -> np.ndarray:
    """[B, T, 2] f32 -> [N_CORES, b_shard, ROW] fp16 (eps plane | dm4 plane)."""
    b = x.shape[0]
    packed = np.empty((b, 2, T), dtype=np.float16)
    packed[:, 0, :] = x[:, :, 0].astype(np.float16)
    packed[:, 1, :] = (-4.0 * x[:, :, 1]).astype(np.float16)
    return packed.reshape(N_CORES, b // N_CORES, ROW)


def run(x: np.ndarray, trace: bool = False):
    """Run the sharded kernel; returns (full_output, BassKernelResults)."""
    b, t_len, c = x.shape
    assert c == 2 and t_len == T and b % N_CORES == 0
    b_shard = b // N_CORES
    shards = _pack(np.asarray(x, dtype=np.float32))
    in_maps = [{"x": np.ascontiguousarray(shards[i])} for i in range(N_CORES)]
    res = run_bass_kernel_spmd(
        _get_nc(b_shard), in_maps,
        core_ids=list(range(N_CORES)), trace=trace,
    )
    out = np.concatenate([r["y"] for r in res.results], axis=0)
    return out.astype(np.float32).reshape(b, t_len, 1), res


def kernel(x: np.ndarray) -> np.ndarray:
    out, _ = run(x, trace=False)
    return out
